# revision 55
# baseline (speedup 1.0000x reference)
"""Trainium2 Bass kernel for nn_AiMAiPartiallyConnectedLayers.

26 independent MLPs (5 -> 64 -> 64 -> 1, tanh) applied per node type over a
batch of 65536 samples; output [B, 26] fp32.  Pure data parallel over 8
NeuronCores (8192 samples each); ~264 us HW time, rel err ~3.6e-3 vs the
fp32 reference (bf16 matmul precision).

Per-core design (ScalarE/tanh is the bottleneck engine; everything else is
structured to hide under it):
  - Types are processed in 13 pairs; pair weights are packed host-side into
    block-diagonal [128, 128] bf16 tiles so one matmul handles 2 types.
  - The input is pre-transposed host-side into a pair-aligned bf16 layout
    xt[tile, q, 128, 512]: pair p = 4q+k occupies partitions 32k:32k+11
    (2x5 channel rows + a ones row that folds b1 into the layer-1 matmul;
    remaining rows zero).  Layer-1 matmuls use row tile_position (32k, 0).
  - Layer-1 tanh is a bias-free ACT op spanning a 2-pair PSUM region
    [128, 2, 512]; layer-2 tanh fuses b2 via the ACT per-partition bias port.
  - Layer-3 weights for pair p sit in a [128, 128] block whose only nonzero
    columns are 32k, 32k+1, so four pairs accumulate into one PSUM bank
    (start/stop flags) with rows pre-zeroed for free; a DVE copy moves the
    bank to SBUF, per-pair SBUF->SBUF DMAs (gpsimd DGE) gather the 26 result
    rows into [26, 512], and PE transposes (identity matmul) restore
    sample-major [512, 26]; DVE adds b3 during the PSUM->SBUF output copy.
  - Two-deep software pipeline: each pair-group's L2/tanh2/L3 tail is
    emitted one group late so ACT1(g+1) and L1(g+2) fill the serial
    ACT1 -> L2 -> ACT2 latency; ps1/ps2 are double-buffered (PSUM exactly
    fills the 8 banks: 2x2 ps1 + 2x1 ps2 + 1 ps3 + 1 psT).

Notes for future edits (hard-won):
  - Build with bacc.Bacc, not bass.Bass: Bacc.finalize() legalizes multi-
    semaphore waits (walrus allows ONE embedded wait per instruction).
  - float32r matmuls measure ~2 cyc/row on HW (947ns at N=512) despite the
    cost model's 1 cyc/row; bf16 hits ~213ns warm (+~170ns SBUF latency).
  - tile_position column offsets (0, 32k) fail walrus' ISA check; only row
    tiling works.  PSUM-source DMA is not allowed (DVE-copy to SBUF first).
  - ACT runs ~0.82ns/elem marginal + ~250ns/op fixed, dtype-independent.
"""

import os
import sys

import numpy as np


def _ensure_path():
    for p in ("/opt/trn_rl_repo",):
        if p not in sys.path:
            sys.path.insert(0, p)


try:
    import concourse.bass as bass  # noqa: F401
except ImportError:
    _ensure_path()

import concourse.bass as bass  # noqa: F401  (AP helpers via bass types)
import concourse.bacc as bacc
import concourse.mybir as mybir
import concourse.tile as tile
from contextlib import ExitStack
from concourse.bass_utils import run_bass_kernel_spmd

NCORES = 8
B = 65536
BC = B // NCORES  # 8192 samples per core
T = 26  # node types
C = 5  # channels
H = 64  # hidden
NPAIR = 13  # type pairs
TILE = 512  # samples per batch tile
NCH = TILE // 128
F32 = mybir.dt.float32
BF16 = mybir.dt.bfloat16
TANH = mybir.ActivationFunctionType.Tanh

# Exposed for test harnesses: last BassKernelResults from a traced run.
LAST_RESULTS = None


def build_nc(bc=BC):
    nt = bc // TILE
    nc = bacc.Bacc("TRN2", target_bir_lowering=False, debug=False)
    xt_d = nc.dram_tensor("xt", [nt, 4, 128, TILE], BF16, kind="ExternalInput")
    w1a_d = nc.dram_tensor("w1a", [128, NPAIR * 128], BF16, kind="ExternalInput")
    w2a_d = nc.dram_tensor("w2a", [128, NPAIR * 128], BF16, kind="ExternalInput")
    w3a_d = nc.dram_tensor("w3a", [128, NPAIR * 128], BF16, kind="ExternalInput")
    b2a_d = nc.dram_tensor("b2a", [128, NPAIR], F32, kind="ExternalInput")
    b3r_d = nc.dram_tensor("b3r", [128, NCH * T], F32, kind="ExternalInput")
    id_d = nc.dram_tensor("ident", [128, 128], F32, kind="ExternalInput")
    out = nc.dram_tensor("out", [bc, T], F32, kind="ExternalOutput")

    with tile.TileContext(nc) as tc, ExitStack() as ctx:
        wpool = ctx.enter_context(tc.tile_pool(name="weights", bufs=1))
        xtpool = ctx.enter_context(tc.tile_pool(name="xt", bufs=4))
        h1pool = ctx.enter_context(tc.tile_pool(name="h1", bufs=5))
        h2pool = ctx.enter_context(tc.tile_pool(name="h2", bufs=4))
        s3pool = ctx.enter_context(tc.tile_pool(name="s3", bufs=nt))
        scpool = ctx.enter_context(tc.tile_pool(name="s3scat", bufs=3))
        opool = ctx.enter_context(tc.tile_pool(name="osb", bufs=nt))
        pp1 = ctx.enter_context(tc.tile_pool(name="ps1", bufs=2, space="PSUM"))
        pp2 = ctx.enter_context(tc.tile_pool(name="ps2", bufs=2, space="PSUM"))
        pp3 = ctx.enter_context(tc.tile_pool(name="ps3", bufs=1, space="PSUM"))
        ppt = ctx.enter_context(tc.tile_pool(name="psT", bufs=1, space="PSUM"))

        # warm the ACT tanh table while the setup DMAs run
        wrm = wpool.tile([1, 1], F32)
        nc.vector.memset(wrm, 0.0)
        nc.scalar.activation(out=wrm, in_=wrm, func=TANH)

        # ------- setup: DMA host-packed constants straight into SBUF.
        # w1a rides the sync queue (first thing layer 1 needs); the rest go
        # via the gpsimd DGE so tile 0's xt load isn't queued behind them.
        w1a = wpool.tile([128, NPAIR * 128], BF16)
        nc.sync.dma_start(out=w1a, in_=w1a_d[:, :])
        ident = wpool.tile([128, 128], F32)
        nc.gpsimd.dma_start(out=ident, in_=id_d[:, :])
        w2a = wpool.tile([128, NPAIR * 128], BF16)
        nc.gpsimd.dma_start(out=w2a, in_=w2a_d[:, :])
        w3a = wpool.tile([128, NPAIR * 128], BF16)
        nc.gpsimd.dma_start(out=w3a, in_=w3a_d[:, :])
        b2a = wpool.tile([128, NPAIR], F32)
        nc.gpsimd.dma_start(out=b2a, in_=b2a_d[:, :])
        b3r = wpool.tile([128, NCH * T], F32)
        nc.gpsimd.dma_start(out=b3r, in_=b3r_d[:, :])

        ngrp = (NPAIR + 1) // 2
        for i in range(nt):
            xt = xtpool.tile([128, 4, TILE], BF16, tag="xt")
            if i == 0:
                nc.sync.dma_start(out=xt[:, 0, :], in_=xt_d[i, 0])
                nc.sync.dma_start(
                    out=xt[:, 1:4, :],
                    in_=xt_d[i, 1:4].rearrange("q p n -> p q n"),
                )
            else:
                nc.sync.dma_start(
                    out=xt, in_=xt_d[i].rearrange("q p n -> p q n")
                )

            def l1_group(g):
                prs = [p for p in (2 * g, 2 * g + 1) if p < NPAIR]
                ps1 = pp1.tile([128, 2, TILE], F32, tag="ps1")
                for j, p in enumerate(prs):
                    q, k = divmod(p, 4)
                    nc.tensor.matmul(
                        out=ps1[:, j, :],
                        lhsT=w1a[32 * k : 32 * k + 32, 128 * p : 128 * (p + 1)],
                        rhs=xt[32 * k : 32 * k + 32, q, :],
                        start=True,
                        stop=True,
                        tile_position=(32 * k, 0),
                    )
                return ps1, prs

            s3 = s3pool.tile([32, TILE], F32, tag="s3")
            state = {"ps3": None}

            def tail(h1, prs):
                for j, p in enumerate(prs):
                    ps2 = pp2.tile([128, TILE], F32, tag="ps2")
                    nc.tensor.matmul(
                        out=ps2,
                        lhsT=w2a[:, 128 * p : 128 * (p + 1)],
                        rhs=h1[:, j, :],
                        start=True,
                        stop=True,
                    )
                    h2 = h2pool.tile([128, TILE], BF16, tag="h2")
                    nc.scalar.activation(
                        out=h2, in_=ps2, func=TANH,
                        bias=b2a[:, p : p + 1], scale=1.0,
                    )
                    q3, k3 = divmod(p, 4)
                    if k3 == 0:
                        state["ps3"] = pp3.tile([128, TILE], F32, tag="ps3", name="ps3")
                    ps3 = state["ps3"]
                    nc.tensor.matmul(
                        out=ps3,
                        lhsT=w3a[:, 128 * p : 128 * (p + 1)],
                        rhs=h2,
                        start=(k3 == 0),
                        stop=(k3 == 3 or p == NPAIR - 1),
                    )
                    if k3 == 3 or p == NPAIR - 1:
                        cnt = k3 + 1
                        s3sc = scpool.tile([128, TILE], F32, tag="s3sc")
                        nc.vector.tensor_copy(
                            out=s3sc[0 : 32 * cnt, :], in_=ps3[0 : 32 * cnt, :]
                        )
                        for k in range(cnt):
                            pp = 4 * q3 + k
                            nc.gpsimd.dma_start(
                                out=s3[2 * pp : 2 * pp + 2, :],
                                in_=s3sc[32 * k : 32 * k + 2, :],
                            )

            # two-deep software pipeline: each group's ACT-dependent tail is
            # emitted one iteration late, so ACT1(g+1) and L1(g+2) fill the
            # serial ACT1(g) -> L2(g) -> ACT2(g) latency.
            groups = {0: l1_group(0)}
            pend = []
            for g in range(ngrp):
                if g + 1 < ngrp:
                    groups[g + 1] = l1_group(g + 1)
                ps1, prs = groups.pop(g)
                gsz = len(prs)
                h1 = h1pool.tile([128, 2, TILE], BF16, tag="h1")
                nc.scalar.activation(
                    out=h1[:, 0:gsz, :], in_=ps1[:, 0:gsz, :], func=TANH
                )
                pend.append((h1, prs))
                if len(pend) > 2:
                    tail(*pend.pop(0))
            for it in pend:
                tail(*it)

            # transpose [26, 512] back to sample-major [128, (c, 26)]
            pst = ppt.tile([128, NCH * T], F32, tag="psT")
            for c in range(NCH):
                nc.tensor.transpose(
                    out=pst[:, T * c : T * (c + 1)],
                    in_=s3[0:T, 128 * c : 128 * (c + 1)],
                    identity=ident[0:T, 0:T],
                )
            osb = opool.tile([128, NCH * T], F32, tag="osb")
            nc.vector.tensor_add(osb, pst, b3r)
            nc.sync.dma_start(
                out=out[i * TILE : (i + 1) * TILE].rearrange(
                    "(c p) t -> p c t", p=128
                ),
                in_=osb.rearrange("p (c t) -> p c t", c=NCH),
            )
    return nc


def pack_weights(W1, b1, W2, b2, W3, b3):
    W1 = np.asarray(W1, dtype=np.float32)
    b1 = np.asarray(b1, dtype=np.float32)
    W2 = np.asarray(W2, dtype=np.float32)
    b2 = np.asarray(b2, dtype=np.float32)
    W3 = np.asarray(W3, dtype=np.float32)
    b3 = np.asarray(b3, dtype=np.float32)
    import ml_dtypes
    bf16 = ml_dtypes.bfloat16
    w1a = np.zeros((128, NPAIR * 128), np.float32)
    w2a = np.zeros((128, NPAIR * 128), np.float32)
    w3a = np.zeros((128, NPAIR * 128), np.float32)
    b2a = np.zeros((128, NPAIR), np.float32)
    for t in range(T):
        p, e = divmod(t, 2)
        k = p % 4
        w1a[32 * k + 5 * e : 32 * k + 5 * e + 5,
            128 * p + 64 * e : 128 * p + 64 * e + 64] = W1[t]
        w1a[32 * k + 10, 128 * p + 64 * e : 128 * p + 64 * e + 64] = b1[t]
        w2a[64 * e : 64 * e + 64, 128 * p + 64 * e : 128 * p + 64 * e + 64] = W2[t]
        w3a[64 * e : 64 * e + 64, 128 * p + 32 * (p % 4) + e] = W3[t][:, 0]
        b2a[64 * e : 64 * e + 64, p] = b2[t]
    b3r = np.ascontiguousarray(np.tile(b3[:, 0], (128, NCH)))
    return {
        "w1a": w1a.astype(bf16),
        "w2a": w2a.astype(bf16),
        "w3a": w3a.astype(bf16),
        "b2a": b2a,
        "b3r": b3r,
        "ident": np.eye(128, dtype=np.float32),
    }


def pack_xt(features_core):
    """[bc, 26, 5] -> [nt, 4, 128, TILE] pair-aligned transposed layout."""
    bc = features_core.shape[0]
    nt = bc // TILE
    ff = np.asarray(features_core, np.float32).reshape(nt, TILE, T, C)
    import ml_dtypes
    xt = np.zeros((nt, 4, 128, TILE), ml_dtypes.bfloat16)
    for t in range(T):
        p, e = divmod(t, 2)
        q, k = divmod(p, 4)
        xt[:, q, 32 * k + 5 * e : 32 * k + 5 * e + 5, :] = ff[:, :, t, :].swapaxes(
            1, 2
        )
    for p in range(NPAIR):
        q, k = divmod(p, 4)
        xt[:, q, 32 * k + 2 * C, :] = 1.0
    return xt


def kernel(features, W1, b1, W2, b2, W3, b3):
    global LAST_RESULTS
    features = np.asarray(features, dtype=np.float32)
    ins = pack_weights(W1, b1, W2, b2, W3, b3)
    nc = build_nc(BC)
    nc.finalize()
    in_maps = []
    for c in range(NCORES):
        m = dict(ins)
        m["xt"] = pack_xt(features[c * BC : (c + 1) * BC])
        in_maps.append(m)
    trace = bool(int(os.environ.get("KERNEL_TRACE", "0")))
    # The first execution of a freshly loaded NEFF intermittently faults with
    # NRT_EXEC_UNIT_UNRECOVERABLE; a retry on the recovered device succeeds.
    last_exc = None
    for attempt in range(3):
        try:
            res = run_bass_kernel_spmd(
                nc, in_maps, list(range(NCORES)), trace=trace
            )
            LAST_RESULTS = res
            return np.concatenate(
                [res.results[c]["out"] for c in range(NCORES)], axis=0
            )
        except Exception as e:  # noqa: BLE001
            last_exc = e
            import time as _time

            _time.sleep(5.0 * (attempt + 1))
    raise last_exc


# revision 56
# speedup vs baseline: 1.0125x; 1.0125x over previous
"""Trainium2 Bass kernel for nn_AiMAiPartiallyConnectedLayers.

26 independent MLPs (5 -> 64 -> 64 -> 1, tanh) applied per node type over a
batch of 65536 samples; output [B, 26] fp32.  Pure data parallel over 8
NeuronCores (8192 samples each); ~264 us HW time, rel err ~3.6e-3 vs the
fp32 reference (bf16 matmul precision).

Per-core design (ScalarE/tanh is the bottleneck engine; everything else is
structured to hide under it):
  - Types are processed in 13 pairs; pair weights are packed host-side into
    block-diagonal [128, 128] bf16 tiles so one matmul handles 2 types.
  - The input is pre-transposed host-side into a pair-aligned bf16 layout
    xt[tile, q, 128, 512]: pair p = 4q+k occupies partitions 32k:32k+11
    (2x5 channel rows + a ones row that folds b1 into the layer-1 matmul;
    remaining rows zero).  Layer-1 matmuls use row tile_position (32k, 0).
  - Layer-1 tanh is a bias-free ACT op spanning a 2-pair PSUM region
    [128, 2, 512]; layer-2 tanh fuses b2 via the ACT per-partition bias port.
  - Layer-3 weights for pair p sit in a [128, 128] block whose only nonzero
    columns are 32k, 32k+1, so four pairs accumulate into one PSUM bank
    (start/stop flags) with rows pre-zeroed for free; a DVE copy moves the
    bank to SBUF, per-pair SBUF->SBUF DMAs (gpsimd DGE) gather the 26 result
    rows into [26, 512], and PE transposes (identity matmul) restore
    sample-major [512, 26]; DVE adds b3 during the PSUM->SBUF output copy.
  - Two-deep software pipeline: each pair-group's L2/tanh2/L3 tail is
    emitted one group late so ACT1(g+1) and L1(g+2) fill the serial
    ACT1 -> L2 -> ACT2 latency; ps1/ps2 are double-buffered (PSUM exactly
    fills the 8 banks: 2x2 ps1 + 2x1 ps2 + 1 ps3 + 1 psT).

Notes for future edits (hard-won):
  - Build with bacc.Bacc, not bass.Bass: Bacc.finalize() legalizes multi-
    semaphore waits (walrus allows ONE embedded wait per instruction).
  - float32r matmuls measure ~2 cyc/row on HW (947ns at N=512) despite the
    cost model's 1 cyc/row; bf16 hits ~213ns warm (+~170ns SBUF latency).
  - tile_position column offsets (0, 32k) fail walrus' ISA check; only row
    tiling works.  PSUM-source DMA is not allowed (DVE-copy to SBUF first).
  - ACT runs ~0.82ns/elem marginal + ~250ns/op fixed, dtype-independent.
"""

import os
import sys

import numpy as np


def _ensure_path():
    for p in ("/opt/trn_rl_repo",):
        if p not in sys.path:
            sys.path.insert(0, p)


try:
    import concourse.bass as bass  # noqa: F401
except ImportError:
    _ensure_path()

import concourse.bass as bass  # noqa: F401  (AP helpers via bass types)
import concourse.bacc as bacc
import concourse.mybir as mybir
import concourse.tile as tile
from contextlib import ExitStack
from concourse.bass_utils import run_bass_kernel_spmd

NCORES = 8
B = 65536
BC = B // NCORES  # 8192 samples per core
T = 26  # node types
C = 5  # channels
H = 64  # hidden
NPAIR = 13  # type pairs
TILE = 512  # samples per batch tile
NCH = TILE // 128
F32 = mybir.dt.float32
BF16 = mybir.dt.bfloat16
TANH = mybir.ActivationFunctionType.Tanh

# Exposed for test harnesses: last BassKernelResults from a traced run.
LAST_RESULTS = None


def build_nc(bc=BC):
    nt = bc // TILE
    nc = bacc.Bacc("TRN2", target_bir_lowering=False, debug=False)
    xt_d = nc.dram_tensor("xt", [nt, 4, 128, TILE], BF16, kind="ExternalInput")
    w1a_d = nc.dram_tensor("w1a", [128, NPAIR * 128], BF16, kind="ExternalInput")
    w2a_d = nc.dram_tensor("w2a", [128, NPAIR * 128], BF16, kind="ExternalInput")
    w3a_d = nc.dram_tensor("w3a", [128, NPAIR * 128], BF16, kind="ExternalInput")
    b2a_d = nc.dram_tensor("b2a", [128, NPAIR], F32, kind="ExternalInput")
    b3r_d = nc.dram_tensor("b3r", [128, NCH * T], F32, kind="ExternalInput")
    id_d = nc.dram_tensor("ident", [128, 128], F32, kind="ExternalInput")
    out = nc.dram_tensor("out", [bc, T], F32, kind="ExternalOutput")

    with tile.TileContext(nc) as tc, ExitStack() as ctx:
        wpool = ctx.enter_context(tc.tile_pool(name="weights", bufs=1))
        xtpool = ctx.enter_context(tc.tile_pool(name="xt", bufs=4))
        h1pool = ctx.enter_context(tc.tile_pool(name="h1", bufs=4))
        h2pool = ctx.enter_context(tc.tile_pool(name="h2", bufs=4))
        s3pool = ctx.enter_context(tc.tile_pool(name="s3", bufs=nt))
        scpool = ctx.enter_context(tc.tile_pool(name="s3scat", bufs=3))
        opool = ctx.enter_context(tc.tile_pool(name="osb", bufs=nt))
        pp1 = ctx.enter_context(tc.tile_pool(name="ps1", bufs=2, space="PSUM"))
        pp2 = ctx.enter_context(tc.tile_pool(name="ps2", bufs=2, space="PSUM"))
        pp3 = ctx.enter_context(tc.tile_pool(name="ps3", bufs=1, space="PSUM"))
        ppt = ctx.enter_context(tc.tile_pool(name="psT", bufs=1, space="PSUM"))

        # warm the ACT tanh table while the setup DMAs run
        wrm = wpool.tile([1, 1], F32)
        nc.vector.memset(wrm, 0.0)
        nc.scalar.activation(out=wrm, in_=wrm, func=TANH)

        # ------- setup: DMA host-packed constants straight into SBUF.
        # w1a rides the sync queue (first thing layer 1 needs); the rest go
        # via the gpsimd DGE so tile 0's xt load isn't queued behind them.
        w1a = wpool.tile([128, NPAIR * 128], BF16)
        nc.sync.dma_start(out=w1a, in_=w1a_d[:, :])
        ident = wpool.tile([128, 128], F32)
        nc.gpsimd.dma_start(out=ident, in_=id_d[:, :])
        w2a = wpool.tile([128, NPAIR * 128], BF16)
        nc.gpsimd.dma_start(out=w2a, in_=w2a_d[:, :])
        w3a = wpool.tile([128, NPAIR * 128], BF16)
        nc.gpsimd.dma_start(out=w3a, in_=w3a_d[:, :])
        b2a = wpool.tile([128, NPAIR], F32)
        nc.gpsimd.dma_start(out=b2a, in_=b2a_d[:, :])
        b3r = wpool.tile([128, NCH * T], F32)
        nc.gpsimd.dma_start(out=b3r, in_=b3r_d[:, :])

        ngrp = (NPAIR + 1) // 2
        for i in range(nt):
            xt = xtpool.tile([128, 4, TILE], BF16, tag="xt")
            if i == 0:
                nc.sync.dma_start(out=xt[:, 0, :], in_=xt_d[i, 0])
                nc.sync.dma_start(
                    out=xt[:, 1:4, :],
                    in_=xt_d[i, 1:4].rearrange("q p n -> p q n"),
                )
            else:
                nc.sync.dma_start(
                    out=xt, in_=xt_d[i].rearrange("q p n -> p q n")
                )

            def l1_group(g):
                prs = [p for p in (2 * g, 2 * g + 1) if p < NPAIR]
                ps1 = pp1.tile([128, 2, TILE], F32, tag="ps1")
                for j, p in enumerate(prs):
                    q, k = divmod(p, 4)
                    nc.tensor.matmul(
                        out=ps1[:, j, :],
                        lhsT=w1a[32 * k : 32 * k + 32, 128 * p : 128 * (p + 1)],
                        rhs=xt[32 * k : 32 * k + 32, q, :],
                        start=True,
                        stop=True,
                        tile_position=(32 * k, 0),
                    )
                return ps1, prs

            s3 = s3pool.tile([32, TILE], F32, tag="s3")
            state = {"ps3": None}

            def tail(h1, prs):
                for j, p in enumerate(prs):
                    ps2 = pp2.tile([128, TILE], F32, tag="ps2")
                    nc.tensor.matmul(
                        out=ps2,
                        lhsT=w2a[:, 128 * p : 128 * (p + 1)],
                        rhs=h1[:, j, :],
                        start=True,
                        stop=True,
                    )
                    h2 = h2pool.tile([128, TILE], BF16, tag="h2")
                    nc.scalar.activation(
                        out=h2, in_=ps2, func=TANH,
                        bias=b2a[:, p : p + 1], scale=1.0,
                    )
                    q3, k3 = divmod(p, 4)
                    if k3 == 0:
                        state["ps3"] = pp3.tile([128, TILE], F32, tag="ps3", name="ps3")
                    ps3 = state["ps3"]
                    nc.tensor.matmul(
                        out=ps3,
                        lhsT=w3a[:, 128 * p : 128 * (p + 1)],
                        rhs=h2,
                        start=(k3 == 0),
                        stop=(k3 == 3 or p == NPAIR - 1),
                    )
                    if k3 == 3 or p == NPAIR - 1:
                        cnt = k3 + 1
                        s3sc = scpool.tile([128, TILE], F32, tag="s3sc")
                        nc.vector.tensor_copy(
                            out=s3sc[0 : 32 * cnt, :], in_=ps3[0 : 32 * cnt, :]
                        )
                        for k in range(cnt):
                            pp = 4 * q3 + k
                            nc.gpsimd.dma_start(
                                out=s3[2 * pp : 2 * pp + 2, :],
                                in_=s3sc[32 * k : 32 * k + 2, :],
                            )

            # two-deep software pipeline: each group's ACT-dependent tail is
            # emitted one iteration late, so ACT1(g+1) and L1(g+2) fill the
            # serial ACT1(g) -> L2(g) -> ACT2(g) latency.
            groups = {0: l1_group(0)}
            pending = None
            for g in range(ngrp):
                if g + 1 < ngrp:
                    groups[g + 1] = l1_group(g + 1)
                ps1, prs = groups.pop(g)
                gsz = len(prs)
                h1 = h1pool.tile([128, 2, TILE], BF16, tag="h1")
                nc.scalar.activation(
                    out=h1[:, 0:gsz, :], in_=ps1[:, 0:gsz, :], func=TANH
                )
                if pending is not None:
                    tail(*pending)
                pending = (h1, prs)
            tail(*pending)

            # transpose [26, 512] back to sample-major [128, (c, 26)]
            pst = ppt.tile([128, NCH * T], F32, tag="psT")
            for c in range(NCH):
                nc.tensor.transpose(
                    out=pst[:, T * c : T * (c + 1)],
                    in_=s3[0:T, 128 * c : 128 * (c + 1)],
                    identity=ident[0:T, 0:T],
                )
            osb = opool.tile([128, NCH * T], F32, tag="osb")
            nc.vector.tensor_add(osb, pst, b3r)
            nc.sync.dma_start(
                out=out[i * TILE : (i + 1) * TILE].rearrange(
                    "(c p) t -> p c t", p=128
                ),
                in_=osb.rearrange("p (c t) -> p c t", c=NCH),
            )
    return nc


def pack_weights(W1, b1, W2, b2, W3, b3):
    W1 = np.asarray(W1, dtype=np.float32)
    b1 = np.asarray(b1, dtype=np.float32)
    W2 = np.asarray(W2, dtype=np.float32)
    b2 = np.asarray(b2, dtype=np.float32)
    W3 = np.asarray(W3, dtype=np.float32)
    b3 = np.asarray(b3, dtype=np.float32)
    import ml_dtypes
    bf16 = ml_dtypes.bfloat16
    w1a = np.zeros((128, NPAIR * 128), np.float32)
    w2a = np.zeros((128, NPAIR * 128), np.float32)
    w3a = np.zeros((128, NPAIR * 128), np.float32)
    b2a = np.zeros((128, NPAIR), np.float32)
    for t in range(T):
        p, e = divmod(t, 2)
        k = p % 4
        w1a[32 * k + 5 * e : 32 * k + 5 * e + 5,
            128 * p + 64 * e : 128 * p + 64 * e + 64] = W1[t]
        w1a[32 * k + 10, 128 * p + 64 * e : 128 * p + 64 * e + 64] = b1[t]
        w2a[64 * e : 64 * e + 64, 128 * p + 64 * e : 128 * p + 64 * e + 64] = W2[t]
        w3a[64 * e : 64 * e + 64, 128 * p + 32 * (p % 4) + e] = W3[t][:, 0]
        b2a[64 * e : 64 * e + 64, p] = b2[t]
    b3r = np.ascontiguousarray(np.tile(b3[:, 0], (128, NCH)))
    return {
        "w1a": w1a.astype(bf16),
        "w2a": w2a.astype(bf16),
        "w3a": w3a.astype(bf16),
        "b2a": b2a,
        "b3r": b3r,
        "ident": np.eye(128, dtype=np.float32),
    }


def pack_xt(features_core):
    """[bc, 26, 5] -> [nt, 4, 128, TILE] pair-aligned transposed layout."""
    bc = features_core.shape[0]
    nt = bc // TILE
    ff = np.asarray(features_core, np.float32).reshape(nt, TILE, T, C)
    import ml_dtypes
    xt = np.zeros((nt, 4, 128, TILE), ml_dtypes.bfloat16)
    for t in range(T):
        p, e = divmod(t, 2)
        q, k = divmod(p, 4)
        xt[:, q, 32 * k + 5 * e : 32 * k + 5 * e + 5, :] = ff[:, :, t, :].swapaxes(
            1, 2
        )
    for p in range(NPAIR):
        q, k = divmod(p, 4)
        xt[:, q, 32 * k + 2 * C, :] = 1.0
    return xt


def kernel(features, W1, b1, W2, b2, W3, b3):
    global LAST_RESULTS
    features = np.asarray(features, dtype=np.float32)
    ins = pack_weights(W1, b1, W2, b2, W3, b3)
    nc = build_nc(BC)
    nc.finalize()
    in_maps = []
    for c in range(NCORES):
        m = dict(ins)
        m["xt"] = pack_xt(features[c * BC : (c + 1) * BC])
        in_maps.append(m)
    trace = bool(int(os.environ.get("KERNEL_TRACE", "0")))
    # The first execution of a freshly loaded NEFF intermittently faults with
    # NRT_EXEC_UNIT_UNRECOVERABLE; a retry on the recovered device succeeds.
    last_exc = None
    for attempt in range(3):
        try:
            res = run_bass_kernel_spmd(
                nc, in_maps, list(range(NCORES)), trace=trace
            )
            LAST_RESULTS = res
            return np.concatenate(
                [res.results[c]["out"] for c in range(NCORES)], axis=0
            )
        except Exception as e:  # noqa: BLE001
            last_exc = e
            import time as _time

            _time.sleep(5.0 * (attempt + 1))
    raise last_exc
